# revision 34
# baseline (speedup 1.0000x reference)
"""AWQ W4A16 linear kernel for Trainium2 (8 NeuronCores, tensor-parallel over N).

out = x @ dequant(qweight, scales, qzeros) + bias
  x:       [8192, 4096]  bf16 (replicated)
  qweight: [512, 14336]  int32 (4-bit packed along K; column-sharded)
  scales:  [64, 14336]   bf16 (column-sharded)
  qzeros:  [64, 1792]    int32 (4-bit packed along N; column-sharded)
  bias:    [14336]       bf16 (column-sharded)
  out:     [8192, 14336] bf16 (gathered from per-core [8192, 1792] shards)
"""
import numpy as np
import ml_dtypes

P = 128
N_CORES = 8
M_FULL, K_FULL, N_FULL, GROUP = 8192, 4096, 14336, 64


def build_nc(M=M_FULL, K=K_FULL, NS=N_FULL // N_CORES, m_chunk=512, n_free=448,
             prol_bufs=2, xt_bufs=2, stage_bufs=2, qrep_bufs=1, shift_bufs=2,
             interleave=False, skip_main=False, skip_prologue=False):
    import concourse.mybir as mybir
    import concourse.tile as tile
    from concourse import bacc

    KT = K // P               # k-tiles of 128 rows
    QROWS_T = P // 8          # qweight rows per k-tile (16)
    NG = K // GROUP           # quantization groups (64)
    NQ = NS // 8              # packed qzeros columns
    NJ = NS // n_free         # output column chunks per psum pass
    MSUB = m_chunk // P       # m-subtiles per chunk
    assert M % m_chunk == 0 and NS % n_free == 0 and K % (GROUP * 2) == 0

    nc = bacc.Bacc("TRN2", target_bir_lowering=False, debug=False)
    dt = mybir.dt
    x = nc.dram_tensor("x", [M, K], dt.bfloat16, kind="ExternalInput")
    qw = nc.dram_tensor("qweight", [K // 8, NS], dt.int32, kind="ExternalInput")
    sc = nc.dram_tensor("scales", [NG, NS], dt.bfloat16, kind="ExternalInput")
    qz = nc.dram_tensor("qzeros", [NG, NQ], dt.int32, kind="ExternalInput")
    bias = nc.dram_tensor("bias", [NS], dt.bfloat16, kind="ExternalInput")
    out = nc.dram_tensor("out", [M, NS], dt.bfloat16, kind="ExternalOutput")

    # constants
    shift_np = (4 * (np.arange(P) % 8)).astype(np.int32).reshape(P, 1)
    shift_dram = nc.inline_tensor(shift_np, name="shiftc")
    # one-hot selector: sel[g, s, p] = 1 iff g == 2s + p//64, so
    # sel[:, s, :].T @ rows gives rows (2s, 2s+1) replicated over partition halves
    sel_np = np.zeros((NG, KT, P), np.float32)
    gg = 2 * np.arange(KT)[:, None] + np.arange(P)[None, :] // GROUP  # [KT, P]
    for s in range(KT):
        sel_np[gg[s], s, np.arange(P)] = 1.0
    sel_dram = nc.inline_tensor(sel_np.astype(ml_dtypes.bfloat16), name="selbf")
    ones_dram = nc.inline_tensor(np.ones((1, P), ml_dtypes.bfloat16), name="ones1")

    def emit_prologue(tc, w_res, bias_rep, shift_col, ones_bf, interleave=None,
                      bias_ps=None):
        rep_bufs = 2
        # interleave: optional (xt0, psums0) — chunk-0/subtile-0 matmuls are
        # emitted right after each k-tile's dequant to fill PE idle time.
        with tc.tile_pool(name="prol", bufs=1) as prol, \
             tc.tile_pool(name="prol2", bufs=prol_bufs) as prol2, \
             tc.tile_pool(name="pps", bufs=1, space="PSUM") as pps:
            scales_sb = prol.tile([NG, NS], dt.bfloat16)
            nc.scalar.dma_start(scales_sb[:], sc[:])
            qz_sb = prol.tile([NG, NQ], dt.int32)
            nc.scalar.dma_start(qz_sb[:], qz[:])
            bias_sb = prol.tile([1, NS], dt.bfloat16)
            nc.scalar.dma_start(bias_sb[:], bias[None, :])
            sel_bf = prol.tile([NG, KT, P], dt.bfloat16)
            nc.scalar.dma_start(sel_bf[:], sel_dram[:])

            # unpack zeros along the free dim: z[g, 8c+j] = (qz[g, c] >> 4j) & 15
            zfull_i = prol.tile([NG, NS], dt.int32)
            zview = zfull_i.rearrange("g (c j) -> g c j", j=8)
            for j in range(8):
                nc.vector.tensor_scalar(
                    zview[:, :, j], qz_sb[:], 4 * j, 15,
                    mybir.AluOpType.logical_shift_right, mybir.AluOpType.bitwise_and,
                )
            zfull = prol.tile([NG, NS], dt.bfloat16)
            nc.gpsimd.tensor_copy(zfull[:], zfull_i[:])

            # bias replicated across partitions (ones outer product); when
            # interleaving, borrow a slot from the interleave psum pool so the
            # prologue fits in 8 PSUM banks total
            for c in range(0, NS, 512):
                w = min(512, NS - c)
                if bias_ps is not None:
                    bp_pool, bp_tag = bias_ps
                    bps = bp_pool.tile([P, 512], dt.float32, tag=bp_tag, name="bps")
                else:
                    bps = pps.tile([P, 512], dt.float32, tag="biasps")
                nc.tensor.matmul(bps[:, :w], ones_bf[:], bias_sb[:, c:c + w],
                                 start=True, stop=True)
                nc.scalar.copy(bias_rep[:, c:c + w], bps[:, :w])

            for sp in range(KT // 2):
                # load qweight rows [32sp, 32sp+32) with 8x partition replication
                # for two k-tiles at once: partition p of [:, t, :] holds
                # qweight row 32sp + 16t + p//8
                q_rep = prol2.tile([P, 2, NS], dt.int32, tag="qrep", bufs=2)
                qv = q_rep.rearrange("(a b) t n -> a b t n", b=8)
                src = qw[2 * QROWS_T * sp:2 * QROWS_T * (sp + 1), :]
                src2 = src.rearrange("(t a) n -> a t n", t=2)
                for b in range(8):
                    nc.scalar.dma_start(qv[:, b, :, :], src2)
                # w4 = (q >> 4*(p%8)) & 15, in place
                shifted_i = q_rep
                nc.vector.tensor_scalar(
                    shifted_i[:], q_rep[:], shift_col[:], 15,
                    mybir.AluOpType.logical_shift_right, mybir.AluOpType.bitwise_and,
                )
                # replicate scales/zeros rows (2s, 2s+1) to partition halves via PE,
                # in column quarters to fit PSUM; then W = (w4 - z) * s
                quarter = NS // 4
                for t in range(2):
                    s = 2 * sp + t
                    for h in range(4):
                        h0 = h * quarter
                        srep = pps.tile([P, quarter], dt.float32, tag="srep", name="srep", bufs=rep_bufs)
                        zrep = pps.tile([P, quarter], dt.float32, tag="zrep", name="zrep", bufs=rep_bufs)
                        nc.tensor.matmul(srep[:], sel_bf[:, s, :], scales_sb[:, h0:h0 + quarter],
                                         start=True, stop=True)
                        nc.tensor.matmul(zrep[:], sel_bf[:, s, :], zfull[:, h0:h0 + quarter],
                                         start=True, stop=True)
                        t1 = prol2.tile([P, quarter], dt.float32, tag="t1", name="t1")
                        nc.vector.tensor_tensor(t1[:], shifted_i[:, t, h0:h0 + quarter], zrep[:],
                                                mybir.AluOpType.subtract)
                        nc.vector.tensor_tensor(w_res[:, s, h0:h0 + quarter], t1[:], srep[:],
                                                mybir.AluOpType.mult)
                    if interleave is not None:
                        xt0, psums0 = interleave
                        for j in range(NJ):
                            nc.tensor.matmul(
                                psums0[j], xt0[:, s, 0:P],
                                w_res[:, s, n_free * j:n_free * (j + 1)],
                                start=(s == 0), stop=(s == KT - 1),
                            )

    def emit_stage_out(stp, psums, m_lo, bias_rep):
        stage = stp.tile([P, NS], dt.bfloat16, tag="stage", name="stage")
        for j in range(NJ):
            nc.vector.tensor_tensor(
                stage[:, n_free * j:n_free * (j + 1)], psums[j],
                bias_rep[:, n_free * j:n_free * (j + 1)], mybir.AluOpType.add,
            )
        nc.scalar.dma_start(out[m_lo:m_lo + P, :], stage[:])

    def emit_main(tc, stp, w_res, bias_rep, chunks, first=None):
        # chunks: list of (m0, mc); first: optional (xt0, mc0) whose
        # subtile-0 was already computed and staged out during the prologue
        with tc.tile_pool(name="xt", bufs=xt_bufs) as xtp, \
             tc.tile_pool(name="mps", bufs=2, space="PSUM") as mps:
            if first is not None:
                xt0, mc0 = first
                for i in range(1, mc0 // P):
                    psums = [
                        mps.tile([P, n_free], dt.float32, tag=f"acc{j}", name=f"acc{j}")
                        for j in range(NJ)
                    ]
                    for s in range(KT):
                        lhsT = xt0[:, s, P * i:P * (i + 1)]
                        for j in range(NJ):
                            nc.tensor.matmul(
                                psums[j], lhsT, w_res[:, s, n_free * j:n_free * (j + 1)],
                                start=(s == 0), stop=(s == KT - 1),
                            )
                    emit_stage_out(stp, psums, P * i, bias_rep)
            for m0, mc in chunks:
                xt = xtp.tile([P, KT, mc], dt.bfloat16, tag=f"xt{mc}", name="xt",
                              bufs=(1 if mc == 256 else xt_bufs))
                for s in range(KT):
                    nc.sync.dma_start(xt[:, s, :], x[m0:m0 + mc, P * s:P * (s + 1)],
                                      transpose=True)
                for i in range(mc // P):
                    psums = [
                        mps.tile([P, n_free], dt.float32, tag=f"acc{j}", name=f"acc{j}")
                        for j in range(NJ)
                    ]
                    for s in range(KT):
                        lhsT = xt[:, s, P * i:P * (i + 1)]
                        for j in range(NJ):
                            nc.tensor.matmul(
                                psums[j], lhsT, w_res[:, s, n_free * j:n_free * (j + 1)],
                                start=(s == 0), stop=(s == KT - 1),
                            )
                    emit_stage_out(stp, psums, m0 + P * i, bias_rep)

    with tile.TileContext(nc) as tc:
        with tc.tile_pool(name="wres", bufs=1) as wres_pool, \
             tc.tile_pool(name="const", bufs=1) as cpool:
            w_res = wres_pool.tile([P, KT, NS], dt.bfloat16)
            bias_rep = cpool.tile([P, NS], dt.bfloat16)
            shift_col = cpool.tile([P, 1], dt.int32)
            ones_bf = cpool.tile([1, P], dt.bfloat16)
            nc.scalar.dma_start(shift_col[:], shift_dram[:])
            nc.scalar.dma_start(ones_bf[:], ones_dram[:])
            if skip_prologue:
                nc.vector.memset(w_res[:], 0.0)
                nc.vector.memset(bias_rep[:], 0.0)
                if not skip_main:
                    chunks = [(i * m_chunk, m_chunk) for i in range(M // m_chunk)]
                    with tc.tile_pool(name="stage", bufs=stage_bufs) as stp:
                        emit_main(tc, stp, w_res, bias_rep, chunks)
            elif skip_main:
                emit_prologue(tc, w_res, bias_rep, shift_col, ones_bf)
            elif interleave and M >= 512 and (M - 512) % 384 == 0:
                mc0 = 256
                chunks = [(256, 256)] + [(512 + i * 384, 384)
                                         for i in range((M - 512) // 384)]
                with tc.tile_pool(name="xt0p", bufs=1) as xtp0, \
                     tc.tile_pool(name="stage", bufs=stage_bufs) as stp:
                    xt0 = xtp0.tile([P, KT, mc0], dt.bfloat16)
                    for s in range(KT):
                        nc.sync.dma_start(xt0[:, s, :], x[0:mc0, P * s:P * (s + 1)],
                                          transpose=True)
                    with tc.tile_pool(name="mps0", bufs=1, space="PSUM") as mps0:
                        psums0 = [
                            mps0.tile([P, n_free], dt.float32, tag=f"iacc{j}", name=f"iacc{j}")
                            for j in range(NJ)
                        ]
                        emit_prologue(tc, w_res, bias_rep, shift_col, ones_bf,
                                      interleave=(xt0, psums0),
                                      bias_ps=(mps0, "iacc0"))
                        emit_stage_out(stp, psums0, 0, bias_rep)
                    emit_main(tc, stp, w_res, bias_rep, chunks, first=(xt0, mc0))
            else:
                emit_prologue(tc, w_res, bias_rep, shift_col, ones_bf)
                chunks = [(i * m_chunk, m_chunk) for i in range(M // m_chunk)]
                with tc.tile_pool(name="stage", bufs=stage_bufs) as stp:
                    emit_main(tc, stp, w_res, bias_rep, chunks)
    nc.compile()
    return nc


def _shard_inputs(inputs):
    ns = N_FULL // N_CORES
    nq = ns // 8
    x = np.asarray(inputs["x"])
    qw = np.asarray(inputs["qweight"])
    sc = np.asarray(inputs["scales"])
    qz = np.asarray(inputs["qzeros"])
    bias = np.asarray(inputs["bias"])
    in_maps = []
    for c in range(N_CORES):
        in_maps.append({
            "x": x,
            "qweight": np.ascontiguousarray(qw[:, c * ns:(c + 1) * ns]),
            "scales": np.ascontiguousarray(sc[:, c * ns:(c + 1) * ns]),
            "qzeros": np.ascontiguousarray(qz[:, c * nq:(c + 1) * nq]),
            "bias": np.ascontiguousarray(bias[c * ns:(c + 1) * ns]),
        })
    return in_maps


_NC_CACHE = {}


def _get_nc():
    if "nc" not in _NC_CACHE:
        _NC_CACHE["nc"] = build_nc()
    return _NC_CACHE["nc"]


def kernel(**inputs) -> np.ndarray:
    from concourse.bass_utils import run_bass_kernel_spmd

    nc = _get_nc()
    in_maps = _shard_inputs(inputs)
    try:
        res = run_bass_kernel_spmd(nc, in_maps, core_ids=list(range(N_CORES)))
    except Exception:
        # transient device/transport failures have been observed; retry once
        import time
        time.sleep(5)
        res = run_bass_kernel_spmd(nc, in_maps, core_ids=list(range(N_CORES)))
    return np.concatenate([res.results[c]["out"] for c in range(N_CORES)], axis=1)


# revision 36
# speedup vs baseline: 4.9910x; 4.9910x over previous
"""AWQ W4A16 linear kernel for Trainium2 (8 NeuronCores, tensor-parallel over N).

out = x @ dequant(qweight, scales, qzeros) + bias
  x:       [8192, 4096]  bf16 (replicated)
  qweight: [512, 14336]  int32 (4-bit packed along K; column-sharded)
  scales:  [64, 14336]   bf16 (column-sharded)
  qzeros:  [64, 1792]    int32 (4-bit packed along N; column-sharded)
  bias:    [14336]       bf16 (column-sharded)
  out:     [8192, 14336] bf16 (gathered from per-core [8192, 1792] shards)
"""
import numpy as np
import ml_dtypes

P = 128
N_CORES = 8
M_FULL, K_FULL, N_FULL, GROUP = 8192, 4096, 14336, 64


def build_nc(M=M_FULL, K=K_FULL, NS=N_FULL // N_CORES, m_chunk=512, n_free=448,
             prol_bufs=2, xt_bufs=2, stage_bufs=2, qrep_bufs=1, shift_bufs=2,
             interleave=False, skip_main=False, skip_prologue=False, repeat=1):
    import concourse.mybir as mybir
    import concourse.tile as tile
    from concourse import bacc

    KT = K // P               # k-tiles of 128 rows
    QROWS_T = P // 8          # qweight rows per k-tile (16)
    NG = K // GROUP           # quantization groups (64)
    NQ = NS // 8              # packed qzeros columns
    NJ = NS // n_free         # output column chunks per psum pass
    MSUB = m_chunk // P       # m-subtiles per chunk
    assert M % m_chunk == 0 and NS % n_free == 0 and K % (GROUP * 2) == 0

    nc = bacc.Bacc("TRN2", target_bir_lowering=False, debug=False)
    dt = mybir.dt
    x = nc.dram_tensor("x", [M, K], dt.bfloat16, kind="ExternalInput")
    qw = nc.dram_tensor("qweight", [K // 8, NS], dt.int32, kind="ExternalInput")
    sc = nc.dram_tensor("scales", [NG, NS], dt.bfloat16, kind="ExternalInput")
    qz = nc.dram_tensor("qzeros", [NG, NQ], dt.int32, kind="ExternalInput")
    bias = nc.dram_tensor("bias", [NS], dt.bfloat16, kind="ExternalInput")
    out = nc.dram_tensor("out", [M, NS], dt.bfloat16, kind="ExternalOutput")

    # constants
    shift_np = (4 * (np.arange(P) % 8)).astype(np.int32).reshape(P, 1)
    shift_dram = nc.inline_tensor(shift_np, name="shiftc")
    # one-hot selector: sel[g, s, p] = 1 iff g == 2s + p//64, so
    # sel[:, s, :].T @ rows gives rows (2s, 2s+1) replicated over partition halves
    sel_np = np.zeros((NG, KT, P), np.float32)
    gg = 2 * np.arange(KT)[:, None] + np.arange(P)[None, :] // GROUP  # [KT, P]
    for s in range(KT):
        sel_np[gg[s], s, np.arange(P)] = 1.0
    sel_dram = nc.inline_tensor(sel_np.astype(ml_dtypes.bfloat16), name="selbf")
    ones_dram = nc.inline_tensor(np.ones((1, P), ml_dtypes.bfloat16), name="ones1")

    def emit_prologue(tc, w_res, bias_rep, shift_col, ones_bf, interleave=None,
                      bias_ps=None):
        rep_bufs = 2
        # interleave: optional (xt0, psums0) — chunk-0/subtile-0 matmuls are
        # emitted right after each k-tile's dequant to fill PE idle time.
        with tc.tile_pool(name="prol", bufs=1) as prol, \
             tc.tile_pool(name="prol2", bufs=prol_bufs) as prol2, \
             tc.tile_pool(name="pps", bufs=1, space="PSUM") as pps:
            scales_sb = prol.tile([NG, NS], dt.bfloat16)
            nc.scalar.dma_start(scales_sb[:], sc[:])
            qz_sb = prol.tile([NG, NQ], dt.int32)
            nc.scalar.dma_start(qz_sb[:], qz[:])
            bias_sb = prol.tile([1, NS], dt.bfloat16)
            nc.scalar.dma_start(bias_sb[:], bias[None, :])
            sel_bf = prol.tile([NG, KT, P], dt.bfloat16)
            nc.scalar.dma_start(sel_bf[:], sel_dram[:])

            # unpack zeros along the free dim: z[g, 8c+j] = (qz[g, c] >> 4j) & 15
            zfull_i = prol.tile([NG, NS], dt.int32)
            zview = zfull_i.rearrange("g (c j) -> g c j", j=8)
            for j in range(8):
                nc.vector.tensor_scalar(
                    zview[:, :, j], qz_sb[:], 4 * j, 15,
                    mybir.AluOpType.logical_shift_right, mybir.AluOpType.bitwise_and,
                )
            zfull = prol.tile([NG, NS], dt.bfloat16)
            nc.gpsimd.tensor_copy(zfull[:], zfull_i[:])

            # bias replicated across partitions (ones outer product); when
            # interleaving, borrow a slot from the interleave psum pool so the
            # prologue fits in 8 PSUM banks total
            for c in range(0, NS, 512):
                w = min(512, NS - c)
                if bias_ps is not None:
                    bp_pool, bp_tag = bias_ps
                    bps = bp_pool.tile([P, 512], dt.float32, tag=bp_tag, name="bps")
                else:
                    bps = pps.tile([P, 512], dt.float32, tag="biasps")
                nc.tensor.matmul(bps[:, :w], ones_bf[:], bias_sb[:, c:c + w],
                                 start=True, stop=True)
                nc.scalar.copy(bias_rep[:, c:c + w], bps[:, :w])

            for sp in range(KT // 2):
                # load qweight rows [32sp, 32sp+32) with 8x partition replication
                # for two k-tiles at once: partition p of [:, t, :] holds
                # qweight row 32sp + 16t + p//8
                q_rep = prol2.tile([P, 2, NS], dt.int32, tag="qrep", bufs=2)
                qv = q_rep.rearrange("(a b) t n -> a b t n", b=8)
                src = qw[2 * QROWS_T * sp:2 * QROWS_T * (sp + 1), :]
                src2 = src.rearrange("(t a) n -> a t n", t=2)
                for b in range(8):
                    nc.scalar.dma_start(qv[:, b, :, :], src2)
                # w4 = (q >> 4*(p%8)) & 15, in place
                shifted_i = q_rep
                nc.vector.tensor_scalar(
                    shifted_i[:], q_rep[:], shift_col[:], 15,
                    mybir.AluOpType.logical_shift_right, mybir.AluOpType.bitwise_and,
                )
                # replicate scales/zeros rows (2s, 2s+1) to partition halves via PE,
                # in column quarters to fit PSUM; then W = (w4 - z) * s
                quarter = NS // 4
                for t in range(2):
                    s = 2 * sp + t
                    for h in range(4):
                        h0 = h * quarter
                        srep = pps.tile([P, quarter], dt.float32, tag="srep", name="srep", bufs=rep_bufs)
                        zrep = pps.tile([P, quarter], dt.float32, tag="zrep", name="zrep", bufs=rep_bufs)
                        nc.tensor.matmul(srep[:], sel_bf[:, s, :], scales_sb[:, h0:h0 + quarter],
                                         start=True, stop=True)
                        nc.tensor.matmul(zrep[:], sel_bf[:, s, :], zfull[:, h0:h0 + quarter],
                                         start=True, stop=True)
                        t1 = prol2.tile([P, quarter], dt.float32, tag="t1", name="t1")
                        nc.vector.tensor_tensor(t1[:], shifted_i[:, t, h0:h0 + quarter], zrep[:],
                                                mybir.AluOpType.subtract)
                        nc.vector.tensor_tensor(w_res[:, s, h0:h0 + quarter], t1[:], srep[:],
                                                mybir.AluOpType.mult)
                    if interleave is not None:
                        xt0, psums0 = interleave
                        for j in range(NJ):
                            nc.tensor.matmul(
                                psums0[j], xt0[:, s, 0:P],
                                w_res[:, s, n_free * j:n_free * (j + 1)],
                                start=(s == 0), stop=(s == KT - 1),
                            )

    def emit_stage_out(stp, psums, m_lo, bias_rep):
        stage = stp.tile([P, NS], dt.bfloat16, tag="stage", name="stage")
        for j in range(NJ):
            nc.vector.tensor_tensor(
                stage[:, n_free * j:n_free * (j + 1)], psums[j],
                bias_rep[:, n_free * j:n_free * (j + 1)], mybir.AluOpType.add,
            )
        nc.scalar.dma_start(out[m_lo:m_lo + P, :], stage[:])

    def emit_main(tc, stp, w_res, bias_rep, chunks, first=None):
        # chunks: list of (m0, mc); first: optional (xt0, mc0) whose
        # subtile-0 was already computed and staged out during the prologue
        with tc.tile_pool(name="xt", bufs=xt_bufs) as xtp, \
             tc.tile_pool(name="mps", bufs=2, space="PSUM") as mps:
            if first is not None:
                xt0, mc0 = first
                for i in range(1, mc0 // P):
                    psums = [
                        mps.tile([P, n_free], dt.float32, tag=f"acc{j}", name=f"acc{j}")
                        for j in range(NJ)
                    ]
                    for s in range(KT):
                        lhsT = xt0[:, s, P * i:P * (i + 1)]
                        for j in range(NJ):
                            nc.tensor.matmul(
                                psums[j], lhsT, w_res[:, s, n_free * j:n_free * (j + 1)],
                                start=(s == 0), stop=(s == KT - 1),
                            )
                    emit_stage_out(stp, psums, P * i, bias_rep)
            for rep in range(repeat):
                for m0, mc in chunks:
                    xt = xtp.tile([P, KT, mc], dt.bfloat16, tag=f"xt{mc}", name="xt",
                                  bufs=(1 if mc == 256 else xt_bufs))
                    for s in range(KT):
                        nc.sync.dma_start(xt[:, s, :], x[m0:m0 + mc, P * s:P * (s + 1)],
                                          transpose=True)
                    for i in range(mc // P):
                        psums = [
                            mps.tile([P, n_free], dt.float32, tag=f"acc{j}", name=f"acc{j}")
                            for j in range(NJ)
                        ]
                        for s in range(KT):
                            lhsT = xt[:, s, P * i:P * (i + 1)]
                            for j in range(NJ):
                                nc.tensor.matmul(
                                    psums[j], lhsT, w_res[:, s, n_free * j:n_free * (j + 1)],
                                    start=(s == 0), stop=(s == KT - 1),
                                )
                        emit_stage_out(stp, psums, m0 + P * i, bias_rep)

    with tile.TileContext(nc) as tc:
        with tc.tile_pool(name="wres", bufs=1) as wres_pool, \
             tc.tile_pool(name="const", bufs=1) as cpool:
            w_res = wres_pool.tile([P, KT, NS], dt.bfloat16)
            bias_rep = cpool.tile([P, NS], dt.bfloat16)
            shift_col = cpool.tile([P, 1], dt.int32)
            ones_bf = cpool.tile([1, P], dt.bfloat16)
            nc.scalar.dma_start(shift_col[:], shift_dram[:])
            nc.scalar.dma_start(ones_bf[:], ones_dram[:])
            if skip_prologue:
                nc.vector.memset(w_res[:], 0.0)
                nc.vector.memset(bias_rep[:], 0.0)
                if not skip_main:
                    chunks = [(i * m_chunk, m_chunk) for i in range(M // m_chunk)]
                    with tc.tile_pool(name="stage", bufs=stage_bufs) as stp:
                        emit_main(tc, stp, w_res, bias_rep, chunks)
            elif skip_main:
                emit_prologue(tc, w_res, bias_rep, shift_col, ones_bf)
            elif interleave and M >= 512 and (M - 512) % 384 == 0:
                mc0 = 256
                chunks = [(256, 256)] + [(512 + i * 384, 384)
                                         for i in range((M - 512) // 384)]
                with tc.tile_pool(name="xt0p", bufs=1) as xtp0, \
                     tc.tile_pool(name="stage", bufs=stage_bufs) as stp:
                    xt0 = xtp0.tile([P, KT, mc0], dt.bfloat16)
                    for s in range(KT):
                        nc.sync.dma_start(xt0[:, s, :], x[0:mc0, P * s:P * (s + 1)],
                                          transpose=True)
                    with tc.tile_pool(name="mps0", bufs=1, space="PSUM") as mps0:
                        psums0 = [
                            mps0.tile([P, n_free], dt.float32, tag=f"iacc{j}", name=f"iacc{j}")
                            for j in range(NJ)
                        ]
                        emit_prologue(tc, w_res, bias_rep, shift_col, ones_bf,
                                      interleave=(xt0, psums0),
                                      bias_ps=(mps0, "iacc0"))
                        emit_stage_out(stp, psums0, 0, bias_rep)
                    emit_main(tc, stp, w_res, bias_rep, chunks, first=(xt0, mc0))
            else:
                emit_prologue(tc, w_res, bias_rep, shift_col, ones_bf)
                chunks = [(i * m_chunk, m_chunk) for i in range(M // m_chunk)]
                with tc.tile_pool(name="stage", bufs=stage_bufs) as stp:
                    emit_main(tc, stp, w_res, bias_rep, chunks)
    nc.compile()
    return nc


def _shard_inputs(inputs):
    ns = N_FULL // N_CORES
    nq = ns // 8
    x = np.asarray(inputs["x"])
    qw = np.asarray(inputs["qweight"])
    sc = np.asarray(inputs["scales"])
    qz = np.asarray(inputs["qzeros"])
    bias = np.asarray(inputs["bias"])
    in_maps = []
    for c in range(N_CORES):
        in_maps.append({
            "x": x,
            "qweight": np.ascontiguousarray(qw[:, c * ns:(c + 1) * ns]),
            "scales": np.ascontiguousarray(sc[:, c * ns:(c + 1) * ns]),
            "qzeros": np.ascontiguousarray(qz[:, c * nq:(c + 1) * nq]),
            "bias": np.ascontiguousarray(bias[c * ns:(c + 1) * ns]),
        })
    return in_maps


_NC_CACHE = {}


def _get_nc():
    if "nc" not in _NC_CACHE:
        _NC_CACHE["nc"] = build_nc()
    return _NC_CACHE["nc"]


def kernel(**inputs) -> np.ndarray:
    from concourse.bass_utils import run_bass_kernel_spmd

    nc = _get_nc()
    in_maps = _shard_inputs(inputs)
    try:
        res = run_bass_kernel_spmd(nc, in_maps, core_ids=list(range(N_CORES)))
    except Exception:
        # transient device/transport failures have been observed; retry once
        import time
        time.sleep(5)
        res = run_bass_kernel_spmd(nc, in_maps, core_ids=list(range(N_CORES)))
    return np.concatenate([res.results[c]["out"] for c in range(N_CORES)], axis=1)
